# revision 8
# baseline (speedup 1.0000x reference)
"""Trainium2 Bass kernel for the Ablock spatial paradigm.

Reference computation (per sample, C=320 channels of 128x128):
    f    = silu(lem(x))
    fatt = lem(sigmoid(f) - 0.5)
    out  = (f + x) * fatt
where lem applies a per-channel circular 1-pixel shift S_c chosen by c%5:
    0: roll -1 along W   1: roll +1 along W
    2: roll -1 along H   3: roll +1 along H   4: identity

Because S_c commutes with elementwise ops and sigmoid(z)-0.5 = 0.5*tanh(z/2):
    u = silu(x);  w = tanh(u/2)
    out = (S u + x) * (0.5 * S^2 w)
so the only data movement is the shifts; silu and tanh share one ACT
table set (silu_and_others) so there are no activation-table reloads.

Sharding: pure data-parallel, one batch sample per NeuronCore (B=8).
Within a core, channels are processed in groups of 16 with the same
shift type: tile [128 (H) x 2048 (16 channels * 128 W)].
  - W shifts fold into the free-dim access patterns of the consuming
    DVE ops (a main op + a tiny wrap-column op).
  - H shifts are cross-partition: done with SBUF->SBUF DMA row shifts.
"""

import numpy as np

import concourse.bacc as bacc
import concourse.mybir as mybir
from concourse.bass_utils import run_bass_kernel_spmd
from concourse.tile import TileContext

B, C, H, W = 8, 320, 128, 128
G = 16  # channels of one shift type per tile
F = G * W  # tile free size
NTYPE_CH = C // 5  # channels per shift type (64)
NCHUNK = NTYPE_CH // G  # tiles per type (4)
FP32 = mybir.dt.float32
AOP = mybir.AluOpType


def _emit(nc, tc, x_d, o_d, f_func=None):
    act = mybir.ActivationFunctionType
    if f_func is None:
        f_func = act.Silu
    with (
        tc.tile_pool(name="xp", bufs=3) as xp,
        tc.tile_pool(name="up", bufs=3) as up,
        tc.tile_pool(name="wp", bufs=2) as wp,
        tc.tile_pool(name="usp", bufs=2) as usp,
        tc.tile_pool(name="wsp", bufs=2) as wsp,
        tc.tile_pool(name="ap", bufs=2) as ap_,
        tc.tile_pool(name="op", bufs=3) as op_,
    ):
        for g in range(NCHUNK):
            for r in range(5):
                c0 = r + 5 * G * g
                src = x_d[c0 : c0 + 5 * (G - 1) + 1 : 5]  # (16,128,128) strided
                xt = xp.tile([H, F], FP32, name="xt")
                nc.sync.dma_start(
                    out=xt.rearrange("p (k w) -> p k w", k=G),
                    in_=src.rearrange("k h w -> h k w"),
                )

                u = up.tile([H, F], FP32, name="u")
                nc.scalar.activation(u, xt, f_func)
                w = wp.tile([H, F], FP32, name="w")
                nc.scalar.activation(w, u, act.Tanh, scale=0.5)

                a = ap_.tile([H, F], FP32, name="a")
                o = op_.tile([H, F], FP32, name="o")
                # 3D views: [partition, channel-in-tile, W]
                x3 = xt.rearrange("p (k j) -> p k j", k=G)
                u3 = u.rearrange("p (k j) -> p k j", k=G)
                w3 = w.rearrange("p (k j) -> p k j", k=G)
                a3 = a.rearrange("p (k j) -> p k j", k=G)
                o3 = o.rearrange("p (k j) -> p k j", k=G)

                if r == 0:
                    # S: out(i,j) = in(i, j+1)
                    nc.vector.tensor_tensor(
                        a3[:, :, 0:127], u3[:, :, 1:128], x3[:, :, 0:127], AOP.add
                    )
                    nc.vector.tensor_tensor(
                        a3[:, :, 127:128], u3[:, :, 0:1], x3[:, :, 127:128], AOP.add
                    )
                    nc.vector.scalar_tensor_tensor(
                        o3[:, :, 0:126], w3[:, :, 2:128], 0.5, a3[:, :, 0:126],
                        AOP.mult, AOP.mult,
                    )
                    nc.vector.scalar_tensor_tensor(
                        o3[:, :, 126:128], w3[:, :, 0:2], 0.5, a3[:, :, 126:128],
                        AOP.mult, AOP.mult,
                    )
                elif r == 1:
                    # S: out(i,j) = in(i, j-1)
                    nc.vector.tensor_tensor(
                        a3[:, :, 1:128], u3[:, :, 0:127], x3[:, :, 1:128], AOP.add
                    )
                    nc.vector.tensor_tensor(
                        a3[:, :, 0:1], u3[:, :, 127:128], x3[:, :, 0:1], AOP.add
                    )
                    nc.vector.scalar_tensor_tensor(
                        o3[:, :, 2:128], w3[:, :, 0:126], 0.5, a3[:, :, 2:128],
                        AOP.mult, AOP.mult,
                    )
                    nc.vector.scalar_tensor_tensor(
                        o3[:, :, 0:2], w3[:, :, 126:128], 0.5, a3[:, :, 0:2],
                        AOP.mult, AOP.mult,
                    )
                elif r == 2 or r == 3:
                    us = usp.tile([H, F], FP32, name="us")
                    ws = wsp.tile([H, F], FP32, name="ws")
                    if r == 2:
                        # S: out(i,:) = in(i+1,:)
                        nc.sync.dma_start(out=us[0:127, :], in_=u[1:128, :])
                        nc.sync.dma_start(out=us[127:128, :], in_=u[0:1, :])
                        nc.sync.dma_start(out=ws[0:126, :], in_=w[2:128, :])
                        nc.sync.dma_start(out=ws[126:128, :], in_=w[0:2, :])
                    else:
                        # S: out(i,:) = in(i-1,:)
                        nc.sync.dma_start(out=us[1:128, :], in_=u[0:127, :])
                        nc.sync.dma_start(out=us[0:1, :], in_=u[127:128, :])
                        nc.sync.dma_start(out=ws[2:128, :], in_=w[0:126, :])
                        nc.sync.dma_start(out=ws[0:2, :], in_=w[126:128, :])
                    nc.vector.tensor_tensor(a, us, xt, AOP.add)
                    nc.vector.scalar_tensor_tensor(o, ws, 0.5, a, AOP.mult, AOP.mult)
                else:
                    nc.vector.tensor_tensor(a, u, xt, AOP.add)
                    nc.vector.scalar_tensor_tensor(o, w, 0.5, a, AOP.mult, AOP.mult)

                dst = o_d[c0 : c0 + 5 * (G - 1) + 1 : 5]
                nc.sync.dma_start(
                    out=dst.rearrange("k h w -> h k w"),
                    in_=o.rearrange("p (k w) -> p k w", k=G),
                )


_NC_CACHE = {}


def _build(f_func=None):
    key = ("nc", str(f_func))
    if key in _NC_CACHE:
        return _NC_CACHE[key]
    nc = bacc.Bacc(
        "TRN2",
        target_bir_lowering=False,
        debug=False,
        enable_asserts=True,
        num_devices=B,
    )
    x_d = nc.dram_tensor("x", [C, H, W], FP32, kind="ExternalInput").ap()
    o_d = nc.dram_tensor("out", [C, H, W], FP32, kind="ExternalOutput").ap()
    with TileContext(nc) as tc:
        _emit(nc, tc, x_d, o_d, f_func=f_func)
    nc.compile()
    _NC_CACHE[key] = nc
    return nc


def run(x, trace=False, tmpdir=None):
    x = np.ascontiguousarray(np.asarray(x), dtype=np.float32)
    assert x.shape == (B, C, H, W), x.shape
    nc = _build()
    in_maps = [{"x": np.ascontiguousarray(x[i])} for i in range(B)]
    res = run_bass_kernel_spmd(
        nc, in_maps, core_ids=list(range(B)), trace=trace, tmpdir=tmpdir
    )
    out = np.stack([res.results[i]["out"] for i in range(B)], axis=0)
    return out, res


def kernel(x):
    out, _ = run(x)
    return out


# revision 15
# speedup vs baseline: 2.8548x; 2.8548x over previous
"""Trainium2 Bass kernel for the Ablock spatial paradigm.

Reference computation (per sample, C=320 channels of 128x128):
    f    = silu(lem(x))
    fatt = lem(sigmoid(f) - 0.5)
    out  = (f + x) * fatt
where lem applies a per-channel circular 1-pixel shift S_c chosen by c%5:
    0: roll -1 along W   1: roll +1 along W
    2: roll -1 along H   3: roll +1 along H   4: identity

Because S_c commutes with elementwise ops and sigmoid(z)-0.5 = 0.5*tanh(z/2):
    u = silu(x);  w = tanh(u/2)
    out = (S u + x) * (0.5 * S^2 w)
so the only data movement is the shifts; silu and tanh share one ACT
table set (silu_and_others) so there are no activation-table reloads.

Sharding: pure data-parallel, one batch sample per NeuronCore (B=8).
Within a core, channels are processed in groups of 16 with the same
shift type: tile [128 (H) x 2048 (16 channels * 128 W)].
  - W shifts fold into the free-dim access patterns of the consuming
    DVE ops (a main op + a tiny wrap-column op).
  - H shifts are cross-partition: done with SBUF->SBUF DMA row shifts.
"""

import numpy as np

import concourse.bacc as bacc
import concourse.mybir as mybir
from concourse.bass_utils import run_bass_kernel_spmd
from concourse.tile import TileContext

B, C, H, W = 8, 320, 128, 128
G = 16  # channels of one shift type per tile
F = G * W  # tile free size
NTYPE_CH = C // 5  # channels per shift type (64)
NCHUNK = NTYPE_CH // G  # tiles per type (4)
FP32 = mybir.dt.float32
AOP = mybir.AluOpType


def _emit(nc, tc, x_d, o_d, p_d, f_func=None):
    act = mybir.ActivationFunctionType
    if f_func is None:
        f_func = act.Silu
    with (
        tc.tile_pool(name="pp", bufs=1) as pp,
        tc.tile_pool(name="xp", bufs=5) as xp,
        tc.tile_pool(name="up", bufs=3) as up,
        tc.tile_pool(name="wp", bufs=2) as wp,
        tc.tile_pool(name="ap", bufs=2) as ap_,
        tc.tile_pool(name="op", bufs=5) as op_,
        tc.tile_pool(name="pup", bufs=4, space="PSUM") as pup,
        tc.tile_pool(name="pwp", bufs=4, space="PSUM") as pwp,
    ):
        # Permutation matrices for the H (cross-partition) circular shifts:
        # perm[d][k][i] = 1 iff k == (i + delta_d) % 128, delta = (+1,+2,-1,-2).
        # As matmul stationary lhsT: (P_d.T @ u)[i,f] = u[(i+delta)%128, f].
        pm = pp.tile([H, 4 * H], FP32, name="pm")
        nc.sync.dma_start(
            out=pm.rearrange("p (d i) -> p d i", d=4),
            in_=p_d.rearrange("d k i -> k d i"),
        )
        pm3 = pm.rearrange("p (d i) -> p d i", d=4)
        for g in range(NCHUNK):
            for r in range(5):
                c0 = r + 5 * G * g
                src = x_d[c0 : c0 + 5 * (G - 1) + 1 : 5]  # (16,128,128) strided
                xt = xp.tile([H, F], FP32, name="xt")
                nc.sync.dma_start(
                    out=xt.rearrange("p (k w) -> p k w", k=G),
                    in_=src.rearrange("k h w -> h k w"),
                )

                u = up.tile([H, F], FP32, name="u")
                nc.scalar.activation(u, xt, f_func)
                w = wp.tile([H, F], FP32, name="w")
                nc.scalar.activation(w, u, act.Tanh, scale=0.5)

                a = ap_.tile([H, F], FP32, name="a")
                o = op_.tile([H, F], FP32, name="o")
                # 3D views: [partition, channel-in-tile, W]
                x3 = xt.rearrange("p (k j) -> p k j", k=G)
                u3 = u.rearrange("p (k j) -> p k j", k=G)
                w3 = w.rearrange("p (k j) -> p k j", k=G)
                a3 = a.rearrange("p (k j) -> p k j", k=G)
                o3 = o.rearrange("p (k j) -> p k j", k=G)

                if r == 0:
                    # S: out(i,j) = in(i, j+1)
                    nc.vector.tensor_tensor(
                        a3[:, :, 0:127], u3[:, :, 1:128], x3[:, :, 0:127], AOP.add
                    )
                    nc.vector.tensor_tensor(
                        a3[:, :, 127:128], u3[:, :, 0:1], x3[:, :, 127:128], AOP.add
                    )
                    nc.vector.scalar_tensor_tensor(
                        o3[:, :, 0:126], w3[:, :, 2:128], 0.5, a3[:, :, 0:126],
                        AOP.mult, AOP.mult,
                    )
                    nc.vector.scalar_tensor_tensor(
                        o3[:, :, 126:128], w3[:, :, 0:2], 0.5, a3[:, :, 126:128],
                        AOP.mult, AOP.mult,
                    )
                elif r == 1:
                    # S: out(i,j) = in(i, j-1)
                    nc.vector.tensor_tensor(
                        a3[:, :, 1:128], u3[:, :, 0:127], x3[:, :, 1:128], AOP.add
                    )
                    nc.vector.tensor_tensor(
                        a3[:, :, 0:1], u3[:, :, 127:128], x3[:, :, 0:1], AOP.add
                    )
                    nc.vector.scalar_tensor_tensor(
                        o3[:, :, 2:128], w3[:, :, 0:126], 0.5, a3[:, :, 2:128],
                        AOP.mult, AOP.mult,
                    )
                    nc.vector.scalar_tensor_tensor(
                        o3[:, :, 0:2], w3[:, :, 126:128], 0.5, a3[:, :, 0:2],
                        AOP.mult, AOP.mult,
                    )
                elif r == 2 or r == 3:
                    # Cross-partition circular shift via permutation matmul
                    # into PSUM: pu = S u, pw = S^2 w (wrap included).
                    p1 = pm3[:, 0, :] if r == 2 else pm3[:, 2, :]
                    p2 = pm3[:, 1, :] if r == 2 else pm3[:, 3, :]
                    NCH = 4  # 512-wide chunks (one PSUM bank each)
                    CW = F // NCH
                    for c in range(NCH):
                        cs = slice(c * CW, (c + 1) * CW)
                        pu = pup.tile([H, CW], FP32, name="pu")
                        nc.tensor.matmul(pu, p1, u[:, cs], start=True, stop=True)
                        nc.vector.tensor_tensor(a[:, cs], pu, xt[:, cs], AOP.add)
                    for c in range(NCH):
                        cs = slice(c * CW, (c + 1) * CW)
                        pw = pwp.tile([H, CW], FP32, name="pw")
                        nc.tensor.matmul(pw, p2, w[:, cs], start=True, stop=True)
                        nc.vector.scalar_tensor_tensor(
                            o[:, cs], pw, 0.5, a[:, cs], AOP.mult, AOP.mult
                        )
                else:
                    nc.vector.tensor_tensor(a, u, xt, AOP.add)
                    nc.vector.scalar_tensor_tensor(o, w, 0.5, a, AOP.mult, AOP.mult)

                dst = o_d[c0 : c0 + 5 * (G - 1) + 1 : 5]
                # Stores on the ACT HWDGE ring so loads (sync ring) and
                # stores generate descriptors in parallel.
                nc.scalar.dma_start(
                    out=dst.rearrange("k h w -> h k w"),
                    in_=o.rearrange("p (k w) -> p k w", k=G),
                )


_NC_CACHE = {}


def _build(f_func=None):
    key = ("nc", str(f_func))
    if key in _NC_CACHE:
        return _NC_CACHE[key]
    nc = bacc.Bacc(
        "TRN2",
        target_bir_lowering=False,
        debug=False,
        enable_asserts=True,
        num_devices=B,
    )
    x_d = nc.dram_tensor("x", [C, H, W], FP32, kind="ExternalInput").ap()
    p_d = nc.dram_tensor("perm", [4, H, H], FP32, kind="ExternalInput").ap()
    o_d = nc.dram_tensor("out", [C, H, W], FP32, kind="ExternalOutput").ap()
    with TileContext(nc) as tc:
        _emit(nc, tc, x_d, o_d, p_d, f_func=f_func)
    nc.compile()
    _NC_CACHE[key] = nc
    return nc


def _perm_mats():
    pm = np.zeros((4, H, H), dtype=np.float32)
    for d, delta in enumerate((1, 2, -1, -2)):
        i = np.arange(H)
        pm[d, (i + delta) % H, i] = 1.0
    return pm


def run(x, trace=False, tmpdir=None):
    x = np.ascontiguousarray(np.asarray(x), dtype=np.float32)
    assert x.shape == (B, C, H, W), x.shape
    nc = _build()
    pm = _perm_mats()
    in_maps = [{"x": np.ascontiguousarray(x[i]), "perm": pm} for i in range(B)]
    res = run_bass_kernel_spmd(
        nc, in_maps, core_ids=list(range(B)), trace=trace, tmpdir=tmpdir
    )
    out = np.stack([res.results[i]["out"] for i in range(B)], axis=0)
    return out, res


def kernel(x):
    out, _ = run(x)
    return out
